# revision 1
# baseline (speedup 1.0000x reference)
"""GNN message-passing kernel for Trainium2 (8 NeuronCores).

out[v] = tanh( sum_w W[w] @ sum_{edges e: v_e=v, widx_e=w} x[u_e] )

Strategy (dest-sharded, no collectives):
  - Nodes (destinations) sharded across 8 cores: core c owns v in
    [c*12500, (c+1)*12500).  Edges bucketed host-side by
    (dest_core, widx, u_window, occurrence_rank) -- sharding/layout step.
  - Per core: Phase Z zeroes per-w segment tables S_w [SRW, D].
    Phase G/S: per (w, u_window): one dma_gather of x[u] rows (int16
    window-local indices) into SBUF staging, then one dma_scatter_add
    per occurrence-rank batch into S_w rows v_local.  Rank batching
    guarantees indices are unique within each scatter instruction
    (HW CCE-add races on duplicates); batches serialize via Tile's
    per-tensor tracking.  Pad slots scatter into a trash row.
    Phase D: per 128-node tile: load S_w tiles, PE-transpose,
    matmul-accumulate against W^T over w, tanh, store.
  - Output: concat of per-core 12500-row slices (host-side unshard).
"""
import os
import numpy as np

import concourse.bass as bass
import concourse.bacc as bacc
import concourse.mybir as mybir
import concourse.tile as tile
from concourse.bass_utils import run_bass_kernel_spmd
from concourse.masks import make_identity

# problem shape (hardcoded per contract)
N, D, E, NW = 100000, 128, 2000000, 8
C = 8                  # cores
NPC = N // C           # 12500 nodes per core
WIN = 32768            # u gather window (int16-addressable rows)
NWIN = 4               # ceil(N / WIN)
SRW = 12800            # S rows per w (98*128=12544 read by dense + trash)
TRASH = 12600          # scatter pad target row (never read by dense phase)
NTILE = 98             # dense-phase node tiles of 128 (12544 rows)

# occurrence-rank batch capacities (multiples of 128), sized from the actual
# seed-0 edge distribution with margin.  win 0-2 see ~10.3k edges per
# (core,w,win); win 3 covers only 1696 source rows.
RANK_CAPS_W012 = [7424, 2816, 896, 256, 128, 128, 128, 128, 128]  # sum 12032
RANK_CAPS_W3 = [640, 128, 128, 128]                               # sum 1024
SLICE_CAPS = [RANK_CAPS_W012] * 3 + [RANK_CAPS_W3]
SLICE_TOT = [sum(cs) for cs in SLICE_CAPS]      # slots per (w, win)
PER_W = sum(SLICE_TOT)                          # 3*12032 + 1024 = 37120
TOT = NW * PER_W                                # slots per core

LAST_RESULTS = None    # BassKernelResults of the most recent run (for profiling)


def _wrap16(flat):
    """[n] -> [128, n/16] idx layout: position i at [i%16, i//16], replicated 8x."""
    base = flat.reshape(-1, 16).T  # [16, n/16]
    return np.tile(base, (8, 1))


def _build_nc():
    nc = bacc.Bacc("TRN2", target_bir_lowering=False, debug=False, num_devices=C,
                   num_swdge_queues=4)
    x_d = nc.dram_tensor("x", [N, D], mybir.dt.float32, kind="ExternalInput")
    wt_d = nc.dram_tensor("wt", [NW, D, D], mybir.dt.float32, kind="ExternalInput")
    gidx_d = nc.dram_tensor("gidx", [128, TOT // 16], mybir.dt.int16, kind="ExternalInput")
    sidx_d = nc.dram_tensor("sidx", [128, TOT // 16], mybir.dt.int16, kind="ExternalInput")
    out_d = nc.dram_tensor("out", [NTILE * 128, D], mybir.dt.float32, kind="ExternalOutput")
    s_w = [nc.dram_tensor(f"S{w}", [SRW, D], mybir.dt.float32) for w in range(NW)]

    with tile.TileContext(nc) as tc:
        # ---- Phase Z: zero all S_w ----
        with tc.tile_pool(name="zpool", bufs=1) as zpool:
            zt = zpool.tile([128, 1600], mybir.dt.float32)
            nc.vector.memset(zt[:], 0.0)
            for w in range(NW):
                sv = s_w[w][:].rearrange("r d -> (r d)").rearrange(
                    "(b p q) -> b p q", p=128, q=1600)
                for b in range(8):
                    nc.sync.dma_start(out=sv[b], in_=zt[:])

            # ---- Phase G/S ----
            with (
                tc.tile_pool(name="idxp", bufs=3) as idxp,
                tc.tile_pool(name="stg", bufs=3) as stg,
            ):
                off = 0
                for w in range(NW):
                    for win in range(NWIN):
                        caps = SLICE_CAPS[win]
                        cap = SLICE_TOT[win]
                        lo, hi = win * WIN, min((win + 1) * WIN, N)
                        gi = idxp.tile([128, cap // 16], mybir.dt.int16, tag="gi")
                        nc.sync.dma_start(
                            out=gi[:], in_=gidx_d[:, off // 16:(off + cap) // 16])
                        si = idxp.tile([128, cap // 16], mybir.dt.int16, tag="si")
                        nc.sync.dma_start(
                            out=si[:], in_=sidx_d[:, off // 16:(off + cap) // 16])
                        st = stg.tile([128, SLICE_TOT[0] // 128, D],
                                      mybir.dt.float32)
                        stv = st[:, :cap // 128, :]
                        nc.gpsimd.dma_gather(
                            stv, x_d[lo:hi], gi[:], cap, cap, D,
                            single_packet=False, queue_num=2 + w % 2)
                        co = 0  # column offset within this slice
                        for bcap in caps:
                            nc.gpsimd.dma_scatter_add(
                                s_w[w][:],
                                st[:, co // 128:(co + bcap) // 128, :],
                                si[:, co // 16:(co + bcap) // 16],
                                bcap, bcap, D,
                                single_packet=False, queue_num=w % 2)
                            co += bcap
                        off += cap

        # ---- Phase D: out = tanh(sum_w S_w @ W_w^T) ----
        with (
            tc.tile_pool(name="const", bufs=1) as constp,
            tc.tile_pool(name="dense", bufs=3) as dense,
            tc.tile_pool(name="psum", bufs=4, space="PSUM") as psum,
        ):
            ident = constp.tile([128, 128], mybir.dt.float32)
            make_identity(nc, ident[:])
            wt = constp.tile([128, NW, D], mybir.dt.float32)
            nc.sync.dma_start(out=wt[:], in_=wt_d[:].rearrange("w j i -> j w i"))
            for t in range(NTILE):
                sload = dense.tile([128, NW, D], mybir.dt.float32)
                for w in range(NW):
                    nc.sync.dma_start(
                        out=sload[:, w, :], in_=s_w[w][t * 128:(t + 1) * 128, :])
                outp = psum.tile([128, 128], mybir.dt.float32, tag="acc")
                for w in range(NW):
                    tp = psum.tile([128, 128], mybir.dt.float32, tag="tp")
                    nc.tensor.transpose(out=tp[:], in_=sload[:, w, :], identity=ident[:])
                    ts = dense.tile([128, 128], mybir.dt.float32, tag="ts")
                    nc.vector.tensor_copy(out=ts[:], in_=tp[:])
                    nc.tensor.matmul(
                        out=outp[:], lhsT=ts[:], rhs=wt[:, w, :],
                        start=(w == 0), stop=(w == NW - 1))
                ot = dense.tile([128, 128], mybir.dt.float32, tag="ot")
                nc.scalar.activation(ot[:], outp[:], mybir.ActivationFunctionType.Tanh)
                nc.sync.dma_start(out=out_d[t * 128:(t + 1) * 128, :], in_=ot[:])

    nc.compile()
    return nc


def _prep_cores(u, v, widx):
    """Bucket edges by (core, w, u_window, occurrence-rank); build idx arrays."""
    c = v // NPC
    uwin = u // WIN
    vloc = v - c * NPC
    bucket = (c * NW + widx) * NWIN + uwin
    # occurrence rank of (bucket, vloc)
    pair = bucket * NPC + vloc
    o1 = np.argsort(pair, kind="stable")
    ps = pair[o1]
    isnew = np.ones(len(ps), bool)
    isnew[1:] = ps[1:] != ps[:-1]
    run_id = np.cumsum(isnew) - 1
    run_starts = np.flatnonzero(isnew)
    rank = np.arange(len(ps)) - run_starts[run_id]
    # order edges by (bucket, rank)
    k2 = bucket[o1] * 16 + rank
    o2 = np.argsort(k2, kind="stable")
    eid = o1[o2]               # edge ids in final order
    k2s = k2[o2]
    # counts per (bucket, rank)
    nb = C * NW * NWIN
    cnt = np.bincount(k2s, minlength=nb * 16).reshape(nb, 16)
    gidx_all, sidx_all = [], []
    gsrc = (u - uwin * WIN).astype(np.int16)
    ssrc = vloc.astype(np.int16)
    pos = np.concatenate([[0], np.cumsum(cnt.reshape(-1))[:-1]]).reshape(nb, 16)
    for cc in range(C):
        g_flat = np.zeros(TOT, np.int16)
        s_flat = np.full(TOT, TRASH, np.int16)
        off = 0
        for w in range(NW):
            for win in range(NWIN):
                caps = SLICE_CAPS[win]
                b = (cc * NW + w) * NWIN + win
                nrank = cnt[b]
                assert nrank[len(caps):].sum() == 0, (
                    f"bucket {b} has ranks beyond {len(caps)}: {nrank}")
                for k, bcap in enumerate(caps):
                    n = int(nrank[k])
                    assert n <= bcap, f"bucket {b} rank {k}: {n} > {bcap}"
                    sel = eid[pos[b, k]:pos[b, k] + n]
                    g_flat[off:off + n] = gsrc[sel]
                    s_flat[off:off + n] = ssrc[sel]
                    off += bcap
        assert off == TOT
        gidx_all.append(_wrap16(g_flat))
        sidx_all.append(_wrap16(s_flat))
    return gidx_all, sidx_all


def kernel(x, W, u, v, widx):
    global LAST_RESULTS
    x = np.ascontiguousarray(np.asarray(x, dtype=np.float32))
    W = np.asarray(W, dtype=np.float32)
    u = np.asarray(u).astype(np.int64)
    v = np.asarray(v).astype(np.int64)
    widx = np.asarray(widx).astype(np.int64)

    gidx_all, sidx_all = _prep_cores(u, v, widx)
    wt_np = np.ascontiguousarray(np.transpose(W, (0, 2, 1)))  # W_T[w] = W[w].T

    nc = _build_nc()
    in_maps = [
        {"x": x, "wt": wt_np, "gidx": gidx_all[cc], "sidx": sidx_all[cc]}
        for cc in range(C)
    ]

    trace = bool(os.environ.get("KERNEL_TRACE"))
    LAST_RESULTS = run_bass_kernel_spmd(
        nc, in_maps, core_ids=list(range(C)),
        trace=trace, trace_cores=[0] if trace else None,
    )
    out = np.concatenate(
        [LAST_RESULTS.results[cc]["out"][:NPC] for cc in range(C)], axis=0)
    return out.astype(np.float32)



# revision 2
# speedup vs baseline: 1.1278x; 1.1278x over previous
"""GNN message-passing kernel v2 for Trainium2 (8 NeuronCores).

out[v] = tanh( sum_w W[w] @ sum_{edges e: v_e=v, widx_e=w} x[u_e] )

Strategy (dest-sharded, PE-based segment sum, no DMA scatter):
  - Nodes (dests) sharded across 8 cores (12500 per core, 98 blocks of 128).
  - x cast to fp16, replicated. Per-edge gather of 256B rows via
    dma_gather, pipelined 4-deep across the 4 SWDGE queues (4 Q7 core
    pairs generate descriptors concurrently; gather g+4 is gated on
    gather g via the idx-tile pool to avoid same-queue overlap).
  - Segment-sum on the TensorEngine: for each 128-slot staging tile and
    each (destblock b, w, window) run overlapping it, a matmul
    S_bw^T += X_tile^T(implicit) @ A_T where A_T[slot, v] is a masked
    one-hot built by DVE is_equal against an iota constant
    (vslot column per incidence, -1 masks pads/other runs).
    Chains accumulate in PSUM chainblocks [128, 8, 128] per block.
  - Dense: out_b = tanh(sum_w S_bw^T.T @ W_w^T) via 8 PSUM-accumulated
    matmuls per block + Tanh on the scalar engine.
  - Edge stream layout: window-major (int16 gather indices address
    32768-row windows), runs ordered (b, w) inside each window, run
    capacities = max over the 8 cores (schedule is SPMD-uniform; the
    per-core variation lives in the gather-index / vslot-column data).
"""
import math
import os
from collections import defaultdict

import numpy as np
import ml_dtypes

import concourse.bass as bass
import concourse.bacc as bacc
import concourse.mybir as mybir
import concourse.tile as tile
from concourse.bass_utils import run_bass_kernel_spmd

# problem shape (hardcoded per contract)
N, D, E, NW = 100000, 128, 2000000, 8
C = 8                  # cores
NPC = N // C           # 12500 dest nodes per core
NB = 98                # dest blocks of 128 (12544 >= 12500)
WINR = 32768           # gather window rows (int16-addressable)
NWIN = 4
CHUNK = 4096           # slots per gather instruction
KB = 32                # one-hot incidences per DVE build op

LAST_RESULTS = None


def _wrap16(flat):
    """[n] -> [128, n/16] idx layout: position i at [i%16, i//16], replicated 8x."""
    base = flat.reshape(-1, 16).T
    return np.tile(base, (8, 1))


def _prep(u, v, widx):
    """Compute the uniform schedule (capacities/incidences) + per-core data."""
    core = v // NPC
    vloc = v - core * NPC
    b = vloc >> 7
    vslot = vloc & 127
    win = u >> 15
    uloc = u & 32767
    w = widx

    key_full = ((core * NB + b) * NW + w) * NWIN + win
    cnt = np.bincount(key_full, minlength=C * NB * NW * NWIN).reshape(
        C, NB, NW, NWIN)
    caps = cnt.max(axis=0).astype(np.int64)          # [NB, NW, NWIN]
    # guarantee every (b, w) chain has >= 1 slot so the chainblock copy
    # never reads uninitialized PSUM
    caps[:, :, 0] = np.maximum(caps[:, :, 0], 1)

    # stream layout: window-major; inside a window: b-major, then w
    run_off = np.zeros((NB, NW, NWIN), np.int64)
    region_start = np.zeros(NWIN, np.int64)
    region_size = np.zeros(NWIN, np.int64)
    nch = np.zeros(NWIN, np.int64)
    off = 0
    for winr in range(NWIN):
        region_start[winr] = off
        cw = caps[:, :, winr].reshape(-1)
        ends = np.cumsum(cw)
        run_off[:, :, winr] = (off + ends - cw).reshape(NB, NW)
        sz = int(ends[-1])
        region_size[winr] = sz
        nch[winr] = (sz + CHUNK - 1) // CHUNK
        off += int(nch[winr]) * CHUNK
    s_total = off

    # block end slot per (b, window) for demand-driven gather emission
    blk_end = run_off[:, NW - 1, :] + caps[:, NW - 1, :]   # [NB, NWIN]

    # incidence list in emission order: (b, win, w, tile).
    # Accumulation chains are per (b, win, w): strictly sequential within
    # each PSUM bank (start=True clears accumulate-flags for the whole
    # 2KB bank, so chains sharing a bank must never interleave).
    inc_meta = []
    chain_lists = defaultdict(list)
    for bb in range(NB):
        for winr in range(NWIN):
            for ww in range(NW):
                o = int(run_off[bb, ww, winr])
                c = int(caps[bb, ww, winr])
                if c == 0:
                    continue
                for t in range(o // 128, (o + c - 1) // 128 + 1):
                    j = len(inc_meta)
                    inc_meta.append((bb, winr, ww, t, o, c))
                    chain_lists[(bb, winr, ww)].append(j)
    ninc = len(inc_meta)
    start_flag = np.zeros(ninc, bool)
    stop_flag = np.zeros(ninc, bool)
    for lst in chain_lists.values():
        start_flag[lst[0]] = True
        stop_flag[lst[-1]] = True
    assert (caps.sum(axis=2) > 0).all()
    nincp = ((ninc + KB - 1) // KB) * KB

    # per-core gather indices + vslot columns
    gidx_all, vcols_all = [], []
    off_by_key = np.transpose(run_off, (2, 0, 1)).reshape(-1)  # [(win,b,w)]
    tile_starts = np.array([t * 128 for (_, _, _, t, _, _) in inc_meta])
    inc_o = np.array([o for (_, _, _, _, o, _) in inc_meta])
    inc_c = np.array([c for (_, _, _, _, _, c) in inc_meta])
    for cc in range(C):
        m = core == cc
        kk = (win[m] * NB + b[m]) * NW + w[m]
        srt = np.lexsort((uloc[m], kk))
        kk_s = kk[srt]
        isnew = np.ones(len(kk_s), bool)
        isnew[1:] = kk_s[1:] != kk_s[:-1]
        starts = np.flatnonzero(isnew)
        rank = np.arange(len(kk_s)) - starts[np.cumsum(isnew) - 1]
        pos = off_by_key[kk_s] + rank
        gflat = np.zeros(s_total, np.int16)
        vflat = np.full(s_total, -1.0, np.float32)
        gflat[pos] = uloc[m][srt].astype(np.int16)
        vflat[pos] = vslot[m][srt]
        gidx_all.append(_wrap16(gflat))

        # vcols[p, j] = vslot of slot tile*128+p if inside run j, else -1
        vt = vflat[(tile_starts[:, None] + np.arange(128)[None, :])]  # [ninc,128]
        pos_in = tile_starts[:, None] + np.arange(128)[None, :]
        mask = (pos_in >= inc_o[:, None]) & (pos_in < (inc_o + inc_c)[:, None])
        vc = np.where(mask, vt, -1.0).astype(np.float32)
        vcp = np.full((nincp, 128), -1.0, np.float32)
        vcp[:ninc] = vc
        vcols_all.append(np.ascontiguousarray(vcp.T).astype(np.float16))

    sched = dict(
        caps=caps, run_off=run_off, region_start=region_start,
        region_size=region_size, nch=nch, blk_end=blk_end, inc_meta=inc_meta,
        start_flag=start_flag, stop_flag=stop_flag,
        ninc=ninc, nincp=nincp, s_total=s_total,
    )
    return sched, gidx_all, vcols_all


def _build_nc(sched):
    nincp = sched["nincp"]
    s_total = sched["s_total"]
    region_start = sched["region_start"]
    nch = sched["nch"]
    blk_end = sched["blk_end"]
    inc_meta = sched["inc_meta"]
    start_flag = sched["start_flag"]
    stop_flag = sched["stop_flag"]
    ninc = sched["ninc"]

    f16 = mybir.dt.float16
    f32 = mybir.dt.float32

    nc = bacc.Bacc("TRN2", target_bir_lowering=False, debug=False,
                   num_devices=C, num_swdge_queues=4)
    x_d = nc.dram_tensor("x", [N, D], f16, kind="ExternalInput")
    wt_d = nc.dram_tensor("wt", [D, NW, D], f16, kind="ExternalInput")
    iota_d = nc.dram_tensor("iota", [128, KB * 128], f16, kind="ExternalInput")
    vcols_d = nc.dram_tensor("vcols", [128, nincp], f16, kind="ExternalInput")
    gidx_d = nc.dram_tensor("gidx", [128, s_total // 16], mybir.dt.int16,
                            kind="ExternalInput")
    out_d = nc.dram_tensor("out", [NB * 128, D], f32, kind="ExternalOutput")

    # group incidences by (block, window) for the emission loop
    incs_by_bwin = defaultdict(list)
    for j, (bb, winr, ww, t, o, c) in enumerate(inc_meta):
        incs_by_bwin[(bb, winr)].append(j)

    with tile.TileContext(nc) as tc:
        with (
            tc.tile_pool(name="const", bufs=1) as constp,
            tc.tile_pool(name="idxp", bufs=4) as idxp,
            tc.tile_pool(name="stg0", bufs=3) as stg0,
            tc.tile_pool(name="stg1", bufs=3) as stg1,
            tc.tile_pool(name="stg2", bufs=3) as stg2,
            tc.tile_pool(name="stg3", bufs=3) as stg3,
            tc.tile_pool(name="atp", bufs=3) as atp,
            tc.tile_pool(name="sstp", bufs=2) as sstp,
            tc.tile_pool(name="outp", bufs=3) as outp,
            tc.tile_pool(name="chains", bufs=2, space="PSUM") as chains,
            tc.tile_pool(name="densep", bufs=3, space="PSUM") as densep,
        ):
            stg = [stg0, stg1, stg2, stg3]
            wt_t = constp.tile([128, NW, D], f16, tag="wt")
            nc.sync.dma_start(
                out=wt_t[:], in_=wt_d[:].rearrange("j w i -> j (w i)").rearrange(
                    "j (w i) -> j w i", w=NW))
            iota_t = constp.tile([128, KB, 128], f16, tag="iota")
            nc.sync.dma_start(
                out=iota_t[:],
                in_=iota_d[:].rearrange("p (k c) -> p k c", k=KB))
            vcols_t = constp.tile([128, nincp], f16, tag="vc")
            nc.sync.dma_start(out=vcols_t[:], in_=vcols_d[:])

            emitted = [0] * NWIN
            stg_tiles = {}
            gcount = [0]

            region_size = sched["region_size"]

            def ensure(winr, upto_slot):
                while region_start[winr] + emitted[winr] * CHUNK < upto_slot:
                    k = emitted[winr]
                    so = int(region_start[winr] + k * CHUNK)
                    left = int(region_size[winr]) - k * CHUNK
                    cap = min(CHUNK, ((left + 127) // 128) * 128)
                    gi = idxp.tile([128, CHUNK // 16], mybir.dt.int16, tag="gi")
                    nc.sync.dma_start(
                        out=gi[:, :cap // 16],
                        in_=gidx_d[:, so // 16:(so + cap) // 16])
                    st = stg[winr].tile([128, CHUNK // 128, D], f16)
                    lo = winr * WINR
                    hi = min(N, lo + WINR)
                    nc.gpsimd.dma_gather(
                        st[:, :cap // 128, :], x_d[lo:hi], gi[:, :cap // 16],
                        cap, cap, D,
                        single_packet=False, queue_num=gcount[0] % 4)
                    stg_tiles[(winr, k)] = st
                    emitted[winr] += 1
                    gcount[0] += 1

            caps = sched["caps"]
            atb = None
            for bb in range(NB):
                ssb_tiles = {}
                for winr in range(NWIN):
                    ensure(winr, int(blk_end[bb, winr]))
                    incs = incs_by_bwin.get((bb, winr), ())
                    if not incs:
                        continue
                    cb = chains.tile([128, NW, 128], f32, tag="cb")
                    for j in incs:
                        _, _, ww, t, o, c = inc_meta[j]
                        if j % KB == 0:
                            atb = atp.tile([128, KB, 128], f16)
                            nc.vector.tensor_tensor(
                                out=atb[:],
                                in0=vcols_t[:, j:j + KB].unsqueeze(2)
                                    .to_broadcast((128, KB, 128)),
                                in1=iota_t[:],
                                op=mybir.AluOpType.is_equal)
                        kch = (t * 128 - int(region_start[winr])) // CHUNK
                        tloc = t - (int(region_start[winr]) + kch * CHUNK) // 128
                        st = stg_tiles[(winr, kch)]
                        nc.tensor.matmul(
                            out=cb[:, ww, :], lhsT=st[:, tloc, :],
                            rhs=atb[:, j % KB, :],
                            start=bool(start_flag[j]), stop=bool(stop_flag[j]))
                    ssb = sstp.tile([128, NW, 128], f16, tag=f"s{winr}")
                    nc.scalar.activation(ssb[:], cb[:],
                                         mybir.ActivationFunctionType.Copy)
                    ssb_tiles[winr] = ssb
                dp = densep.tile([128, 128], f32, tag="d")
                terms = [(ww, winr) for ww in range(NW) for winr in range(NWIN)
                         if caps[bb, ww, winr] > 0]
                for i, (ww, winr) in enumerate(terms):
                    nc.tensor.matmul(
                        out=dp[:], lhsT=ssb_tiles[winr][:, ww, :],
                        rhs=wt_t[:, ww, :],
                        start=(i == 0), stop=(i == len(terms) - 1))
                ot = outp.tile([128, 128], f32, tag="o")
                nc.scalar.activation(ot[:], dp[:],
                                     mybir.ActivationFunctionType.Tanh)
                nc.sync.dma_start(out=out_d[bb * 128:(bb + 1) * 128, :],
                                  in_=ot[:])

    # Align each gather's SWDGE queue with its Tile-assigned DMASW sem lane
    # (lane k -> queue k%4). Mixed queues on one lane would let completions
    # arrive out of order vs the lane's tick counter, releasing waiters
    # early; alignment also guarantees any 4 consecutively-scheduled
    # gathers use 4 distinct queues (and same-queue gathers are FIFO).
    from concourse.tile_scheduler import PROC_NAME_TO_IDX
    sw_lane = {PROC_NAME_TO_IDX[f"DMASW{k}"]: k for k in range(8)}
    for f in nc.m.functions:
        for blk in f.blocks:
            for inst in blk.instructions:
                if isinstance(inst, mybir.InstDMAGatherAnt):
                    lane = sw_lane.get(inst.bass_scheduled_proc)
                    if lane is not None:
                        inst.queue_num = lane % 4

    nc.compile()
    return nc


def kernel(x, W, u, v, widx):
    global LAST_RESULTS
    x = np.ascontiguousarray(np.asarray(x, dtype=np.float32))
    W = np.asarray(W, dtype=np.float32)
    u = np.asarray(u).astype(np.int64)
    v = np.asarray(v).astype(np.int64)
    widx = np.asarray(widx).astype(np.int64)

    sched, gidx_all, vcols_all = _prep(u, v, widx)
    x16 = x.astype(np.float16)
    # wt[j, w, i] = W[w, i, j]
    wt_np = np.ascontiguousarray(W.transpose(2, 0, 1)).astype(np.float16)
    iota_np = np.ascontiguousarray(
        np.broadcast_to(
            np.tile(np.arange(128, dtype=np.float32), KB)[None, :],
            (128, KB * 128))).astype(np.float16)

    nc = _build_nc(sched)
    in_maps = [
        {"x": x16, "wt": wt_np, "iota": iota_np,
         "vcols": vcols_all[cc], "gidx": gidx_all[cc]}
        for cc in range(C)
    ]

    trace = bool(os.environ.get("KERNEL_TRACE"))
    LAST_RESULTS = run_bass_kernel_spmd(
        nc, in_maps, core_ids=list(range(C)),
        trace=trace, trace_cores=[0] if trace else None,
    )
    out = np.concatenate(
        [np.asarray(LAST_RESULTS.results[cc]["out"])[:NPC] for cc in range(C)],
        axis=0)
    return out.astype(np.float32)
